# revision 31
# baseline (speedup 1.0000x reference)
"""Gated max/avg 2x2 pooling kernel for Trainium2 (8 NeuronCores, SPMD).

Reference computation (per 2x2 window over [B, H, W, C], stride 2):
    x1 = max(window), x2 = mean(window)
    xs = sum_ij mask[i, j] * window[i, j]   (per channel)
    z  = sigmoid(xs)
    out = z * x1 + (1 - z) * x2

Sharding: pure data-parallel over batch (16 batches -> 2 per core).

Design notes (v2, derived from perfetto engine calibration):
  - DVE tensor_tensor runs 2x in fp16, tensor_scalar 4x, but
    scalar_tensor_tensor only 1x; ACT is 1x for everything.  So the
    kernel uses only TT + one TS on DVE and moves all scalings to ACT.
  - x is cast to fp16 on the HOST: halves input DMA and eliminates the
    on-device f32->fp16 cast that dominated the ACT engine.
  - conv path: xs = f * (u0*Ee + u1*Eo + u2*Oe + u3*Oo), u_k = m_k/f,
    f = mask value of max |.|, so |u_k| <= 1.  The u_k scalings run on
    ACT (per-partition scalar APs), the adds on DVE, and f rides the
    sigmoid's free scale.  Program is mask-independent.
  - combine: s4 = 0.25*s (DVE TS 4x); d = x1 - s4; g = z*d; o = s4 + g.
  - output stored as fp16, host casts back to f32 (tolerance 2e-2).

Per-core layout: partition dim = 128 output rows; one macro-tile =
(batch, w-quarter); free dim = (w_pair 32, even/odd 2, channel 64).
"""

import numpy as np

import concourse.bacc as bacc
import concourse.mybir as mybir
import concourse.tile as tile
from concourse.bass_utils import run_bass_kernel_spmd

F32 = mybir.dt.float32
F16 = mybir.dt.float16

B, H, W, C = 16, 256, 256, 64
N_CORES = 8
BPC = B // N_CORES          # batches per core
HO = H // 2                 # 128 output rows = SBUF partitions
NQ = 4                      # w-quarters per row
WQ = W // NQ                # input w per macro-tile (64)

# Set by kernel() when tracing is enabled (env KERNEL_TRACE=1).
LAST_EXEC_NS = None
LAST_RESULTS = None

_PROGRAM_CACHE = {}


def _build_program(bpc, ho, nq, wq, ch):
    """Build + compile the single-core Bass/Tile program (SPMD-shared)."""
    from contextlib import ExitStack

    assert ho == 128, "partition dim must be 128"
    fd_in = wq * ch            # free dim of an E/O row tile (4096)
    wo = wq // 2               # output w per macro-tile (32)

    nc = bacc.Bacc(
        "TRN2",
        target_bir_lowering=False,
        debug=False,
        enable_asserts=False,
        num_devices=N_CORES,
    )

    x = nc.dram_tensor("x", [bpc, ho, 2, nq, fd_in], F16, kind="ExternalInput")
    scal = nc.dram_tensor("scal", [128, 8], F32, kind="ExternalInput")
    out = nc.dram_tensor(
        "out", [bpc, ho, nq, wo * ch], F16, kind="ExternalOutput"
    )
    x_ap = x.ap()
    out_ap = out.ap()

    alu = mybir.AluOpType

    with tile.TileContext(nc) as tc, ExitStack() as ctx:
        pool_io = ctx.enter_context(tc.tile_pool(name="io", bufs=2))
        pool_u = ctx.enter_context(tc.tile_pool(name="upool", bufs=2))
        pool_big = ctx.enter_context(tc.tile_pool(name="big", bufs=2))
        pool_tmp = ctx.enter_context(tc.tile_pool(name="tmp", bufs=2))
        pool_out = ctx.enter_context(tc.tile_pool(name="outp", bufs=2))
        pool_const = ctx.enter_context(tc.tile_pool(name="const", bufs=1))

        # scal rides the ACT HWDGE ring so the first x-tile load is the
        # head of the sync ring's FIFO.
        scal_t = pool_const.tile([128, 8], F32)
        nc.scalar.dma_start(scal_t[:], scal.ap()[:])
        u_aps = [scal_t[:, k : k + 1] for k in range(4)]
        f_ap = scal_t[:, 4:5]

        def emit_warmup():
            # Warm the sigmoid ACT table set during the first DMA; the
            # table loads then overlap the first tile's transfer instead
            # of landing on the critical path.
            warm = pool_const.tile([128, 1], F32)
            nc.scalar.activation(
                warm[:], scal_t[:, 7:8],
                mybir.ActivationFunctionType.Sigmoid, bias=0.0, scale=1.0,
            )

        def emit_load(b, q, w_lo, w_hi, eng=None, post_dma=None):
            """Stage 1: DMA the tile + the 4 ACT scaled copies for the
            conv path (issued one tile ahead so ACT leads DVE).

            The very first load goes out on the ACT HWDGE ring (eng=
            nc.scalar): the Sync engine spends the first ~8us of the
            NEFF in the all-core startup barrier, while the ACT ring is
            free from ~2.5us."""
            nw = w_hi - w_lo
            fde = nw * 2 * ch
            fdo = nw * ch
            EO = pool_io.tile([128, 2 * fde], F16, tag="EO")
            src = x_ap[b, :, :, q, :].rearrange(
                "p r (w c) -> p r w c", c=2 * ch
            )[:, :, w_lo : w_lo + nw, :]
            (eng or nc.sync).dma_start(
                EO[:].rearrange("p (r w c) -> p r w c", r=2, c=2 * ch), src
            )
            if post_dma is not None:
                post_dma()
            EO4 = EO[:].rearrange("p (r w e c) -> p r w e c", r=2, e=2, c=ch)
            terms = [
                EO4[:, 0, :, 0, :], EO4[:, 0, :, 1, :],
                EO4[:, 1, :, 0, :], EO4[:, 1, :, 1, :],
            ]
            U = pool_u.tile([128, 4 * fdo], F16, tag="U")
            U4 = U[:].rearrange("p (k w c) -> p k w c", k=4, c=ch)
            for k in range(4):
                nc.scalar.mul(U4[:, k], terms[k], u_aps[k])
            return dict(b=b, q=q, w_lo=w_lo, nw=nw, fde=fde, fdo=fdo,
                        EO=EO, U=U)

        def emit_compute(h, first=False):
            """Stage 2: DVE TT ops + sigmoid + output DMA for one tile.

            Steady-state DVE order [conv adds, max/sum, combine] keeps the
            sigmoid -> g dependency off the critical path.  For the first
            tile the max/sum ops go first: they depend only on the DMA,
            not on ACT's U copies, so DVE starts ~2us earlier."""
            b, q, w_lo, nw = h["b"], h["q"], h["w_lo"], h["nw"]
            fde, fdo, EO, Uf = h["fde"], h["fdo"], h["EO"], h["U"]

            def tmp3(tag, pool=pool_tmp, fd=fdo):
                t = pool.tile([128, fd], F16, tag=tag)
                return t, t[:].rearrange("p (w c) -> p w c", c=ch)

            Ef = EO[:, 0:fde].rearrange("p (w c) -> p w c", c=ch)
            Of = EO[:, fde : 2 * fde].rearrange("p (w c) -> p w c", c=ch)

            def sum_part():
                # SA = [S1 de-interleaved (e,w,c) | a12]: vertical sum
                # written e-major so vs_e/vs_o are contiguous 2048-runs,
                # matching the contiguous a1/a2 written next to them; the
                # second-level adds of both paths then fuse into ONE
                # double-width TT ([s | t3]) below.
                SA = pool_big.tile([128, fde + 2 * fdo], F16, tag="SA")
                nc.vector.tensor_add(
                    SA[:, 0:fde].rearrange("p (e w c) -> p e w c", e=2, c=ch),
                    EO[:, 0:fde].rearrange("p (w e c) -> p e w c", e=2, c=ch),
                    EO[:, fde : 2 * fde].rearrange(
                        "p (w e c) -> p e w c", e=2, c=ch
                    ),
                )
                return SA

            def conv_finish(SA):
                # conv pair partials [a1 | a2] (one TT over the U tile
                # viewed [pair 2, elem 2, fdo])
                Upe = Uf[:].rearrange("p (k e f) -> p k e f", k=2, e=2)
                nc.vector.tensor_add(
                    SA[:, fde : fde + 2 * fdo].rearrange(
                        "p (k f) -> p k f", k=2
                    ),
                    Upe[:, :, 0], Upe[:, :, 1],
                )
                # fused second level: [s | t3] in one TT add
                SAv = SA[:].rearrange(
                    "p (seg half f) -> p seg half f", seg=2, half=2
                )
                FT, _ = tmp3("FT", fd=2 * fdo)
                nc.vector.tensor_add(
                    FT[:].rearrange("p (seg f) -> p seg f", seg=2),
                    SAv[:, :, 0], SAv[:, :, 1],
                )
                t3v = FT[:, fdo : 2 * fdo].rearrange("p (w c) -> p w c", c=ch)
                z, zv = tmp3("z")
                nc.scalar.activation(
                    zv, t3v, mybir.ActivationFunctionType.Sigmoid,
                    bias=0.0, scale=f_ap,
                )
                s4, s4v = tmp3("s4")
                nc.vector.tensor_scalar_mul(
                    s4v, FT[:, 0:fdo].rearrange("p (w c) -> p w c", c=ch),
                    0.25,
                )
                return zv, s4v

            def max_pool():
                # max pool: vertical max (full width), then horizontal pairs
                M1, M1v = tmp3("M1", pool_big, fd=fde)
                nc.vector.tensor_max(M1v, Ef, Of)
                M13 = M1[:, 0:fde].rearrange(
                    "p (w e c) -> p w e c", e=2, c=ch
                )
                x1, x1v = tmp3("x1")
                nc.vector.tensor_max(x1v, M13[:, :, 0, :], M13[:, :, 1, :])
                return x1v

            if first:
                # ramp tiles: run everything U-independent (max pool +
                # vertical sum) before the conv ops so the DVE is not
                # gated on ACT's U copies while ACT is still catching up
                x1v = max_pool()
                SA = sum_part()
                zv, s4v = conv_finish(SA)
            else:
                SA = sum_part()
                zv, s4v = conv_finish(SA)
                x1v = max_pool()

            # gating: out = s4 + z*(x1 - s4)
            d, dv = tmp3("d")
            nc.vector.tensor_tensor(dv, x1v, s4v, alu.subtract)
            g, gv = tmp3("g")
            nc.vector.tensor_mul(gv, zv, dv)
            o, ov = tmp3("o", pool_out)
            nc.vector.tensor_add(ov, s4v, gv)

            # stores ride the ACT HWDGE ring: they never queue behind the
            # (much larger) input loads on the sync ring.
            dst = out_ap[b, :, q, :].rearrange("p (w c) -> p w c", c=ch)
            nc.scalar.dma_start(
                dst[:, w_lo : w_lo + nw, :],
                o[:].rearrange("p (w c) -> p w c", c=ch),
            )

        wo_q = wq // 2  # output w-pairs per quarter (32)
        n_tiles = bpc * nq
        tiles = []
        for b in range(bpc):
            for q in range(nq):
                idx = b * nq + q
                if idx == 0 and wo_q >= 8:
                    # graduated first tiles: cut the startup stall and
                    # keep ACT's U-copy chunks small while it ramps
                    tiles.append((b, q, 0, wo_q // 4))
                    tiles.append((b, q, wo_q // 4, wo_q // 2))
                    tiles.append((b, q, wo_q // 2, wo_q))
                elif idx == n_tiles - 1 and wo_q >= 8:
                    # split the last tile so the final store is small
                    tiles.append((b, q, 0, 3 * wo_q // 4))
                    tiles.append((b, q, 3 * wo_q // 4, wo_q))
                else:
                    tiles.append((b, q, 0, wo_q))
        emit_warmup()
        pending = emit_load(*tiles[0])
        for i in range(len(tiles)):
            nxt = emit_load(*tiles[i + 1]) if i + 1 < len(tiles) else None
            emit_compute(pending, first=(i <= 4))
            pending = nxt

    nc.compile()
    return nc


def _get_program(bpc, ho, nq, wq, ch):
    key = (bpc, ho, nq, wq, ch)
    if key not in _PROGRAM_CACHE:
        _PROGRAM_CACHE[key] = _build_program(bpc, ho, nq, wq, ch)
    return _PROGRAM_CACHE[key]


def _mask_scalars(mask):
    """Per-partition scalar tensor [128, 8] for the conv path.

    xs = f * (u0*Ee + u1*Eo + u2*Oe + u3*Oo) with u_k = m_k / f and
    f = the mask entry of largest magnitude (signed), so |u_k| <= 1.
    f == 0 implies all-zero mask -> z = sigmoid(0) = 0.5 everywhere.
    """
    m = np.asarray(mask, np.float64).reshape(-1)  # m00, m01, m10, m11
    f = m[int(np.argmax(np.abs(m)))]
    u = m / f if f != 0.0 else np.zeros(4)
    scal = np.zeros((128, 8), np.float32)
    scal[:, 0:4] = u.astype(np.float32)
    scal[:, 4] = f
    return scal


def kernel(x, mask):
    import os

    global LAST_EXEC_NS, LAST_RESULTS

    x = np.asarray(x)
    mask = np.asarray(mask)
    assert x.shape == (B, H, W, C), x.shape
    in_dtype = x.dtype

    scal = _mask_scalars(mask)
    nc = _get_program(BPC, HO, NQ, WQ, C)

    xv = x.astype(np.float16).reshape(B, HO, 2, NQ, WQ * C)

    in_maps = [
        {"x": xv[i * BPC : (i + 1) * BPC], "scal": scal} for i in range(N_CORES)
    ]

    trace = os.environ.get("KERNEL_TRACE", "0") == "1"
    res = run_bass_kernel_spmd(
        nc, in_maps, core_ids=list(range(N_CORES)), trace=trace
    )
    LAST_EXEC_NS = res.exec_time_ns
    LAST_RESULTS = res

    parts = [
        r["out"].reshape(BPC, HO, NQ, WQ // 2, C).reshape(BPC, HO, W // 2, C)
        for r in res.results
    ]
    full = np.concatenate(parts, axis=0)
    return full.astype(np.float32, copy=False).astype(in_dtype, copy=False)


def _numpy_reference(x, mask):
    xr = x.reshape(x.shape[0], x.shape[1] // 2, 2, x.shape[2] // 2, 2, x.shape[3])
    x1 = xr.max(axis=(2, 4))
    x2 = xr.mean(axis=(2, 4))
    xs = np.einsum("bhiwjc,ij->bhwc", xr, mask)
    z = 1.0 / (1.0 + np.exp(-xs))
    return z * x1 + (1.0 - z) * x2


if __name__ == "__main__":
    # Small-scale CoreSim self-test (no hardware needed).
    from concourse.bass_interp import CoreSim

    rng = np.random.default_rng(0)
    for bpc_s, nq_s, wq_s in [(1, 1, 8), (1, 2, 32)]:
        h_s, w_s = 256, nq_s * wq_s
        xs_np = rng.standard_normal((bpc_s, h_s, w_s, C)).astype(np.float32)
        mask_np = (rng.standard_normal((2, 2)) * 0.5).astype(np.float32)

        scal_s = _mask_scalars(mask_np)
        nc = _build_program(bpc_s, 128, nq_s, wq_s, C)
        sim = CoreSim(nc, trace=False)
        xv_s = xs_np.astype(np.float16).reshape(bpc_s, 128, 2, nq_s, wq_s * C)
        sim.tensor("x")[:] = xv_s
        sim.tensor("scal")[:] = scal_s
        sim.simulate()
        got = (
            sim.tensor("out")
            .astype(np.float32)
            .reshape(bpc_s, 128, nq_s, wq_s // 2, C)
            .reshape(bpc_s, 128, w_s // 2, C)
        )
        want = _numpy_reference(
            xs_np.astype(np.float64), mask_np.astype(np.float64)
        )
        err = np.abs(got - want)
        rel = err.max() / np.abs(want).max()
        print(f"CoreSim selftest ({nq_s=} {wq_s=}): abs {err.max():.2e} rel {rel:.2e}")
        assert rel < 5e-3, rel
    print("PASS")


# revision 33
# speedup vs baseline: 1.0085x; 1.0085x over previous
"""Gated max/avg 2x2 pooling kernel for Trainium2 (8 NeuronCores, SPMD).

Reference computation (per 2x2 window over [B, H, W, C], stride 2):
    x1 = max(window), x2 = mean(window)
    xs = sum_ij mask[i, j] * window[i, j]   (per channel)
    z  = sigmoid(xs)
    out = z * x1 + (1 - z) * x2

Sharding: pure data-parallel over batch (16 batches -> 2 per core).

Design notes (v2, derived from perfetto engine calibration):
  - DVE tensor_tensor runs 2x in fp16, tensor_scalar 4x, but
    scalar_tensor_tensor only 1x; ACT is 1x for everything.  So the
    kernel uses only TT + one TS on DVE and moves all scalings to ACT.
  - x is cast to fp16 on the HOST: halves input DMA and eliminates the
    on-device f32->fp16 cast that dominated the ACT engine.
  - conv path: xs = f * (u0*Ee + u1*Eo + u2*Oe + u3*Oo), u_k = m_k/f,
    f = mask value of max |.|, so |u_k| <= 1.  The u_k scalings run on
    ACT (per-partition scalar APs), the adds on DVE, and f rides the
    sigmoid's free scale.  Program is mask-independent.
  - combine: s4 = 0.25*s (DVE TS 4x); d = x1 - s4; g = z*d; o = s4 + g.
  - output stored as fp16, host casts back to f32 (tolerance 2e-2).

Per-core layout: partition dim = 128 output rows; one macro-tile =
(batch, w-quarter); free dim = (w_pair 32, even/odd 2, channel 64).
"""

import numpy as np

import concourse.bacc as bacc
import concourse.mybir as mybir
import concourse.tile as tile
from concourse.bass_utils import run_bass_kernel_spmd

F32 = mybir.dt.float32
F16 = mybir.dt.float16

B, H, W, C = 16, 256, 256, 64
N_CORES = 8
BPC = B // N_CORES          # batches per core
HO = H // 2                 # 128 output rows = SBUF partitions
NQ = 4                      # w-quarters per row
WQ = W // NQ                # input w per macro-tile (64)

# Set by kernel() when tracing is enabled (env KERNEL_TRACE=1).
LAST_EXEC_NS = None
LAST_RESULTS = None

_PROGRAM_CACHE = {}


def _build_program(bpc, ho, nq, wq, ch):
    """Build + compile the single-core Bass/Tile program (SPMD-shared)."""
    from contextlib import ExitStack

    assert ho == 128, "partition dim must be 128"
    fd_in = wq * ch            # free dim of an E/O row tile (4096)
    wo = wq // 2               # output w per macro-tile (32)

    nc = bacc.Bacc(
        "TRN2",
        target_bir_lowering=False,
        debug=False,
        enable_asserts=False,
        num_devices=N_CORES,
    )

    x = nc.dram_tensor("x", [bpc, ho, 2, nq, fd_in], F16, kind="ExternalInput")
    scal = nc.dram_tensor("scal", [128, 8], F32, kind="ExternalInput")
    out = nc.dram_tensor(
        "out", [bpc, ho, nq, wo * ch], F16, kind="ExternalOutput"
    )
    x_ap = x.ap()
    out_ap = out.ap()

    alu = mybir.AluOpType

    with tile.TileContext(nc) as tc, ExitStack() as ctx:
        pool_io = ctx.enter_context(tc.tile_pool(name="io", bufs=2))
        pool_u = ctx.enter_context(tc.tile_pool(name="upool", bufs=2))
        pool_big = ctx.enter_context(tc.tile_pool(name="big", bufs=2))
        pool_tmp = ctx.enter_context(tc.tile_pool(name="tmp", bufs=2))
        pool_out = ctx.enter_context(tc.tile_pool(name="outp", bufs=2))
        pool_const = ctx.enter_context(tc.tile_pool(name="const", bufs=1))

        # scal rides the ACT HWDGE ring so the first x-tile load is the
        # head of the sync ring's FIFO.
        scal_t = pool_const.tile([128, 8], F32)
        nc.scalar.dma_start(scal_t[:], scal.ap()[:])
        u_aps = [scal_t[:, k : k + 1] for k in range(4)]
        f_ap = scal_t[:, 4:5]

        def emit_warmup():
            # Warm the sigmoid ACT table set during the first DMA; the
            # table loads then overlap the first tile's transfer instead
            # of landing on the critical path.
            warm = pool_const.tile([128, 1], F32)
            nc.scalar.activation(
                warm[:], scal_t[:, 7:8],
                mybir.ActivationFunctionType.Sigmoid, bias=0.0, scale=1.0,
            )

        def emit_load(b, q, w_lo, w_hi, eng=None, post_dma=None):
            """Stage 1: DMA the tile + the 4 ACT scaled copies for the
            conv path (issued one tile ahead so ACT leads DVE).

            The very first load goes out on the ACT HWDGE ring (eng=
            nc.scalar): the Sync engine spends the first ~8us of the
            NEFF in the all-core startup barrier, while the ACT ring is
            free from ~2.5us."""
            nw = w_hi - w_lo
            fde = nw * 2 * ch
            fdo = nw * ch
            EO = pool_io.tile([128, 2 * fde], F16, tag="EO")
            src = x_ap[b, :, :, q, :].rearrange(
                "p r (w c) -> p r w c", c=2 * ch
            )[:, :, w_lo : w_lo + nw, :]
            (eng or nc.sync).dma_start(
                EO[:].rearrange("p (r w c) -> p r w c", r=2, c=2 * ch), src
            )
            if post_dma is not None:
                post_dma()
            EO4 = EO[:].rearrange("p (r w e c) -> p r w e c", r=2, e=2, c=ch)
            terms = [
                EO4[:, 0, :, 0, :], EO4[:, 0, :, 1, :],
                EO4[:, 1, :, 0, :], EO4[:, 1, :, 1, :],
            ]
            U = pool_u.tile([128, 4 * fdo], F16, tag="U")
            U4 = U[:].rearrange("p (k w c) -> p k w c", k=4, c=ch)
            for k in range(4):
                nc.scalar.mul(U4[:, k], terms[k], u_aps[k])
            return dict(b=b, q=q, w_lo=w_lo, nw=nw, fde=fde, fdo=fdo,
                        EO=EO, U=U)

        def emit_compute(h, first=False):
            """Stage 2: DVE TT ops + sigmoid + output DMA for one tile.

            Steady-state DVE order [conv adds, max/sum, combine] keeps the
            sigmoid -> g dependency off the critical path.  For the first
            tile the max/sum ops go first: they depend only on the DMA,
            not on ACT's U copies, so DVE starts ~2us earlier."""
            b, q, w_lo, nw = h["b"], h["q"], h["w_lo"], h["nw"]
            fde, fdo, EO, Uf = h["fde"], h["fdo"], h["EO"], h["U"]

            def tmp3(tag, pool=pool_tmp, fd=fdo):
                t = pool.tile([128, fd], F16, tag=tag)
                return t, t[:].rearrange("p (w c) -> p w c", c=ch)

            Ef = EO[:, 0:fde].rearrange("p (w c) -> p w c", c=ch)
            Of = EO[:, fde : 2 * fde].rearrange("p (w c) -> p w c", c=ch)

            def sum_part():
                # SA = [S1 de-interleaved (e,w,c) | a12]: vertical sum
                # written e-major so vs_e/vs_o are contiguous 2048-runs,
                # matching the contiguous a1/a2 written next to them; the
                # second-level adds of both paths then fuse into ONE
                # double-width TT ([s | t3]) below.
                SA = pool_big.tile([128, fde + 2 * fdo], F16, tag="SA")
                nc.vector.tensor_add(
                    SA[:, 0:fde].rearrange("p (e w c) -> p e w c", e=2, c=ch),
                    EO[:, 0:fde].rearrange("p (w e c) -> p e w c", e=2, c=ch),
                    EO[:, fde : 2 * fde].rearrange(
                        "p (w e c) -> p e w c", e=2, c=ch
                    ),
                )
                return SA

            def conv_finish(SA, split_a=False):
                # conv pair partials [a1 | a2]: one fused TT in steady
                # state; on ramp tiles two separate adds, so a1 can start
                # after only TWO of ACT's four U copies have landed
                if split_a:
                    nc.vector.tensor_add(
                        SA[:, fde : fde + fdo],
                        Uf[:, 0:fdo], Uf[:, fdo : 2 * fdo],
                    )
                    nc.vector.tensor_add(
                        SA[:, fde + fdo : fde + 2 * fdo],
                        Uf[:, 2 * fdo : 3 * fdo], Uf[:, 3 * fdo : 4 * fdo],
                    )
                else:
                    Upe = Uf[:].rearrange("p (k e f) -> p k e f", k=2, e=2)
                    nc.vector.tensor_add(
                        SA[:, fde : fde + 2 * fdo].rearrange(
                            "p (k f) -> p k f", k=2
                        ),
                        Upe[:, :, 0], Upe[:, :, 1],
                    )
                # fused second level: [s | t3] in one TT add
                SAv = SA[:].rearrange(
                    "p (seg half f) -> p seg half f", seg=2, half=2
                )
                FT, _ = tmp3("FT", fd=2 * fdo)
                nc.vector.tensor_add(
                    FT[:].rearrange("p (seg f) -> p seg f", seg=2),
                    SAv[:, :, 0], SAv[:, :, 1],
                )
                t3v = FT[:, fdo : 2 * fdo].rearrange("p (w c) -> p w c", c=ch)
                z, zv = tmp3("z")
                nc.scalar.activation(
                    zv, t3v, mybir.ActivationFunctionType.Sigmoid,
                    bias=0.0, scale=f_ap,
                )
                s4, s4v = tmp3("s4")
                nc.vector.tensor_scalar_mul(
                    s4v, FT[:, 0:fdo].rearrange("p (w c) -> p w c", c=ch),
                    0.25,
                )
                return zv, s4v

            def max_pool():
                # max pool: vertical max (full width), then horizontal pairs
                M1, M1v = tmp3("M1", pool_big, fd=fde)
                nc.vector.tensor_max(M1v, Ef, Of)
                M13 = M1[:, 0:fde].rearrange(
                    "p (w e c) -> p w e c", e=2, c=ch
                )
                x1, x1v = tmp3("x1")
                nc.vector.tensor_max(x1v, M13[:, :, 0, :], M13[:, :, 1, :])
                return x1v

            if first:
                # ramp tiles: run everything U-independent (max pool +
                # vertical sum) before the conv ops so the DVE is not
                # gated on ACT's U copies while ACT is still catching up
                x1v = max_pool()
                SA = sum_part()
                zv, s4v = conv_finish(SA, split_a=True)
            else:
                SA = sum_part()
                zv, s4v = conv_finish(SA)
                x1v = max_pool()

            # gating: out = s4 + z*(x1 - s4)
            d, dv = tmp3("d")
            nc.vector.tensor_tensor(dv, x1v, s4v, alu.subtract)
            g, gv = tmp3("g")
            nc.vector.tensor_mul(gv, zv, dv)
            o, ov = tmp3("o", pool_out)
            nc.vector.tensor_add(ov, s4v, gv)

            # stores ride the ACT HWDGE ring: they never queue behind the
            # (much larger) input loads on the sync ring.
            dst = out_ap[b, :, q, :].rearrange("p (w c) -> p w c", c=ch)
            nc.scalar.dma_start(
                dst[:, w_lo : w_lo + nw, :],
                o[:].rearrange("p (w c) -> p w c", c=ch),
            )

        wo_q = wq // 2  # output w-pairs per quarter (32)
        n_tiles = bpc * nq
        tiles = []
        for b in range(bpc):
            for q in range(nq):
                idx = b * nq + q
                if idx == 0 and wo_q >= 8:
                    # graduated first tiles: cut the startup stall
                    tiles.append((b, q, 0, wo_q // 4))
                    tiles.append((b, q, wo_q // 4, wo_q))
                elif idx == n_tiles - 1 and wo_q >= 8:
                    # split the last tile so the final store is small
                    tiles.append((b, q, 0, 3 * wo_q // 4))
                    tiles.append((b, q, 3 * wo_q // 4, wo_q))
                else:
                    tiles.append((b, q, 0, wo_q))
        emit_warmup()
        pending = emit_load(*tiles[0])
        for i in range(len(tiles)):
            nxt = emit_load(*tiles[i + 1]) if i + 1 < len(tiles) else None
            emit_compute(pending, first=(i <= 3))
            pending = nxt

    nc.compile()
    return nc


def _get_program(bpc, ho, nq, wq, ch):
    key = (bpc, ho, nq, wq, ch)
    if key not in _PROGRAM_CACHE:
        _PROGRAM_CACHE[key] = _build_program(bpc, ho, nq, wq, ch)
    return _PROGRAM_CACHE[key]


def _mask_scalars(mask):
    """Per-partition scalar tensor [128, 8] for the conv path.

    xs = f * (u0*Ee + u1*Eo + u2*Oe + u3*Oo) with u_k = m_k / f and
    f = the mask entry of largest magnitude (signed), so |u_k| <= 1.
    f == 0 implies all-zero mask -> z = sigmoid(0) = 0.5 everywhere.
    """
    m = np.asarray(mask, np.float64).reshape(-1)  # m00, m01, m10, m11
    f = m[int(np.argmax(np.abs(m)))]
    u = m / f if f != 0.0 else np.zeros(4)
    scal = np.zeros((128, 8), np.float32)
    scal[:, 0:4] = u.astype(np.float32)
    scal[:, 4] = f
    return scal


def kernel(x, mask):
    import os

    global LAST_EXEC_NS, LAST_RESULTS

    x = np.asarray(x)
    mask = np.asarray(mask)
    assert x.shape == (B, H, W, C), x.shape
    in_dtype = x.dtype

    scal = _mask_scalars(mask)
    nc = _get_program(BPC, HO, NQ, WQ, C)

    xv = x.astype(np.float16).reshape(B, HO, 2, NQ, WQ * C)

    in_maps = [
        {"x": xv[i * BPC : (i + 1) * BPC], "scal": scal} for i in range(N_CORES)
    ]

    trace = os.environ.get("KERNEL_TRACE", "0") == "1"
    res = run_bass_kernel_spmd(
        nc, in_maps, core_ids=list(range(N_CORES)), trace=trace
    )
    LAST_EXEC_NS = res.exec_time_ns
    LAST_RESULTS = res

    parts = [
        r["out"].reshape(BPC, HO, NQ, WQ // 2, C).reshape(BPC, HO, W // 2, C)
        for r in res.results
    ]
    full = np.concatenate(parts, axis=0)
    return full.astype(np.float32, copy=False).astype(in_dtype, copy=False)


def _numpy_reference(x, mask):
    xr = x.reshape(x.shape[0], x.shape[1] // 2, 2, x.shape[2] // 2, 2, x.shape[3])
    x1 = xr.max(axis=(2, 4))
    x2 = xr.mean(axis=(2, 4))
    xs = np.einsum("bhiwjc,ij->bhwc", xr, mask)
    z = 1.0 / (1.0 + np.exp(-xs))
    return z * x1 + (1.0 - z) * x2


if __name__ == "__main__":
    # Small-scale CoreSim self-test (no hardware needed).
    from concourse.bass_interp import CoreSim

    rng = np.random.default_rng(0)
    for bpc_s, nq_s, wq_s in [(1, 1, 8), (1, 2, 32)]:
        h_s, w_s = 256, nq_s * wq_s
        xs_np = rng.standard_normal((bpc_s, h_s, w_s, C)).astype(np.float32)
        mask_np = (rng.standard_normal((2, 2)) * 0.5).astype(np.float32)

        scal_s = _mask_scalars(mask_np)
        nc = _build_program(bpc_s, 128, nq_s, wq_s, C)
        sim = CoreSim(nc, trace=False)
        xv_s = xs_np.astype(np.float16).reshape(bpc_s, 128, 2, nq_s, wq_s * C)
        sim.tensor("x")[:] = xv_s
        sim.tensor("scal")[:] = scal_s
        sim.simulate()
        got = (
            sim.tensor("out")
            .astype(np.float32)
            .reshape(bpc_s, 128, nq_s, wq_s // 2, C)
            .reshape(bpc_s, 128, w_s // 2, C)
        )
        want = _numpy_reference(
            xs_np.astype(np.float64), mask_np.astype(np.float64)
        )
        err = np.abs(got - want)
        rel = err.max() / np.abs(want).max()
        print(f"CoreSim selftest ({nq_s=} {wq_s=}): abs {err.max():.2e} rel {rel:.2e}")
        assert rel < 5e-3, rel
    print("PASS")


# revision 34
# speedup vs baseline: 1.0169x; 1.0084x over previous
"""Gated max/avg 2x2 pooling kernel for Trainium2 (8 NeuronCores, SPMD).

Reference computation (per 2x2 window over [B, H, W, C], stride 2):
    x1 = max(window), x2 = mean(window)
    xs = sum_ij mask[i, j] * window[i, j]   (per channel)
    z  = sigmoid(xs)
    out = z * x1 + (1 - z) * x2

Sharding: pure data-parallel over batch (16 batches -> 2 per core).

Design notes (v2, derived from perfetto engine calibration):
  - DVE tensor_tensor runs 2x in fp16, tensor_scalar 4x, but
    scalar_tensor_tensor only 1x; ACT is 1x for everything.  So the
    kernel uses only TT + one TS on DVE and moves all scalings to ACT.
  - x is cast to fp16 on the HOST: halves input DMA and eliminates the
    on-device f32->fp16 cast that dominated the ACT engine.
  - conv path: xs = f * (u0*Ee + u1*Eo + u2*Oe + u3*Oo), u_k = m_k/f,
    f = mask value of max |.|, so |u_k| <= 1.  The u_k scalings run on
    ACT (per-partition scalar APs), the adds on DVE, and f rides the
    sigmoid's free scale.  Program is mask-independent.
  - combine: s4 = 0.25*s (DVE TS 4x); d = x1 - s4; g = z*d; o = s4 + g.
  - output stored as fp16, host casts back to f32 (tolerance 2e-2).

Per-core layout: partition dim = 128 output rows; one macro-tile =
(batch, w-quarter); free dim = (w_pair 32, even/odd 2, channel 64).
"""

import numpy as np

import concourse.bacc as bacc
import concourse.mybir as mybir
import concourse.tile as tile
from concourse.bass_utils import run_bass_kernel_spmd

F32 = mybir.dt.float32
F16 = mybir.dt.float16

B, H, W, C = 16, 256, 256, 64
N_CORES = 8
BPC = B // N_CORES          # batches per core
HO = H // 2                 # 128 output rows = SBUF partitions
NQ = 4                      # w-quarters per row
WQ = W // NQ                # input w per macro-tile (64)

# Set by kernel() when tracing is enabled (env KERNEL_TRACE=1).
LAST_EXEC_NS = None
LAST_RESULTS = None

_PROGRAM_CACHE = {}


def _build_program(bpc, ho, nq, wq, ch):
    """Build + compile the single-core Bass/Tile program (SPMD-shared)."""
    from contextlib import ExitStack

    assert ho == 128, "partition dim must be 128"
    fd_in = wq * ch            # free dim of an E/O row tile (4096)
    wo = wq // 2               # output w per macro-tile (32)

    nc = bacc.Bacc(
        "TRN2",
        target_bir_lowering=False,
        debug=False,
        enable_asserts=False,
        num_devices=N_CORES,
    )

    x = nc.dram_tensor("x", [bpc, ho, 2, nq, fd_in], F16, kind="ExternalInput")
    scal = nc.dram_tensor("scal", [128, 8], F32, kind="ExternalInput")
    out = nc.dram_tensor(
        "out", [bpc, ho, nq, wo * ch], F16, kind="ExternalOutput"
    )
    x_ap = x.ap()
    out_ap = out.ap()

    alu = mybir.AluOpType

    with tile.TileContext(nc) as tc, ExitStack() as ctx:
        pool_io = ctx.enter_context(tc.tile_pool(name="io", bufs=2))
        pool_u = ctx.enter_context(tc.tile_pool(name="upool", bufs=2))
        pool_big = ctx.enter_context(tc.tile_pool(name="big", bufs=2))
        pool_tmp = ctx.enter_context(tc.tile_pool(name="tmp", bufs=2))
        pool_out = ctx.enter_context(tc.tile_pool(name="outp", bufs=2))
        pool_const = ctx.enter_context(tc.tile_pool(name="const", bufs=1))

        # scal rides the ACT HWDGE ring so the first x-tile load is the
        # head of the sync ring's FIFO.
        scal_t = pool_const.tile([128, 8], F32)
        nc.scalar.dma_start(scal_t[:], scal.ap()[:])
        u_aps = [scal_t[:, k : k + 1] for k in range(4)]
        f_ap = scal_t[:, 4:5]

        def emit_warmup():
            # Warm the sigmoid ACT table set during the first DMA; the
            # table loads then overlap the first tile's transfer instead
            # of landing on the critical path.
            warm = pool_const.tile([128, 1], F32)
            nc.scalar.activation(
                warm[:], scal_t[:, 7:8],
                mybir.ActivationFunctionType.Sigmoid, bias=0.0, scale=1.0,
            )

        def emit_load(b, q, w_lo, w_hi, eng=None, post_dma=None):
            """Stage 1: DMA the tile + the 4 ACT scaled copies for the
            conv path (issued one tile ahead so ACT leads DVE).

            The very first load goes out on the ACT HWDGE ring (eng=
            nc.scalar): the Sync engine spends the first ~8us of the
            NEFF in the all-core startup barrier, while the ACT ring is
            free from ~2.5us."""
            nw = w_hi - w_lo
            fde = nw * 2 * ch
            fdo = nw * ch
            EO = pool_io.tile([128, 2 * fde], F16, tag="EO")
            src = x_ap[b, :, :, q, :].rearrange(
                "p r (w c) -> p r w c", c=2 * ch
            )[:, :, w_lo : w_lo + nw, :]
            (eng or nc.sync).dma_start(
                EO[:].rearrange("p (r w c) -> p r w c", r=2, c=2 * ch), src
            )
            if post_dma is not None:
                post_dma()
            EO4 = EO[:].rearrange("p (r w e c) -> p r w e c", r=2, e=2, c=ch)
            terms = [
                EO4[:, 0, :, 0, :], EO4[:, 0, :, 1, :],
                EO4[:, 1, :, 0, :], EO4[:, 1, :, 1, :],
            ]
            U = pool_u.tile([128, 4 * fdo], F16, tag="U")
            U4 = U[:].rearrange("p (k w c) -> p k w c", k=4, c=ch)
            for k in range(4):
                nc.scalar.mul(U4[:, k], terms[k], u_aps[k])
            return dict(b=b, q=q, w_lo=w_lo, nw=nw, fde=fde, fdo=fdo,
                        EO=EO, U=U)

        def emit_compute(h, first=False):
            """Stage 2: DVE TT ops + sigmoid + output DMA for one tile.

            Steady-state DVE order [conv adds, max/sum, combine] keeps the
            sigmoid -> g dependency off the critical path.  For the first
            tile the max/sum ops go first: they depend only on the DMA,
            not on ACT's U copies, so DVE starts ~2us earlier."""
            b, q, w_lo, nw = h["b"], h["q"], h["w_lo"], h["nw"]
            fde, fdo, EO, Uf = h["fde"], h["fdo"], h["EO"], h["U"]

            def tmp3(tag, pool=pool_tmp, fd=fdo):
                t = pool.tile([128, fd], F16, tag=tag)
                return t, t[:].rearrange("p (w c) -> p w c", c=ch)

            Ef = EO[:, 0:fde].rearrange("p (w c) -> p w c", c=ch)
            Of = EO[:, fde : 2 * fde].rearrange("p (w c) -> p w c", c=ch)

            def sum_part():
                # SA = [S1 de-interleaved (e,w,c) | a12]: vertical sum
                # written e-major so vs_e/vs_o are contiguous 2048-runs,
                # matching the contiguous a1/a2 written next to them; the
                # second-level adds of both paths then fuse into ONE
                # double-width TT ([s | t3]) below.
                SA = pool_big.tile([128, fde + 2 * fdo], F16, tag="SA")
                nc.vector.tensor_add(
                    SA[:, 0:fde].rearrange("p (e w c) -> p e w c", e=2, c=ch),
                    EO[:, 0:fde].rearrange("p (w e c) -> p e w c", e=2, c=ch),
                    EO[:, fde : 2 * fde].rearrange(
                        "p (w e c) -> p e w c", e=2, c=ch
                    ),
                )
                return SA

            def conv_finish(SA):
                # conv pair partials [a1 | a2] (one TT over the U tile
                # viewed [pair 2, elem 2, fdo])
                Upe = Uf[:].rearrange("p (k e f) -> p k e f", k=2, e=2)
                nc.vector.tensor_add(
                    SA[:, fde : fde + 2 * fdo].rearrange(
                        "p (k f) -> p k f", k=2
                    ),
                    Upe[:, :, 0], Upe[:, :, 1],
                )
                # fused second level: [s | t3] in one TT add
                SAv = SA[:].rearrange(
                    "p (seg half f) -> p seg half f", seg=2, half=2
                )
                FT, _ = tmp3("FT", fd=2 * fdo)
                nc.vector.tensor_add(
                    FT[:].rearrange("p (seg f) -> p seg f", seg=2),
                    SAv[:, :, 0], SAv[:, :, 1],
                )
                t3v = FT[:, fdo : 2 * fdo].rearrange("p (w c) -> p w c", c=ch)
                z, zv = tmp3("z")
                nc.scalar.activation(
                    zv, t3v, mybir.ActivationFunctionType.Sigmoid,
                    bias=0.0, scale=f_ap,
                )
                s4, s4v = tmp3("s4")
                nc.vector.tensor_scalar_mul(
                    s4v, FT[:, 0:fdo].rearrange("p (w c) -> p w c", c=ch),
                    0.25,
                )
                return zv, s4v

            def max_pool():
                # max pool: vertical max (full width), then horizontal pairs
                M1, M1v = tmp3("M1", pool_big, fd=fde)
                nc.vector.tensor_max(M1v, Ef, Of)
                M13 = M1[:, 0:fde].rearrange(
                    "p (w e c) -> p w e c", e=2, c=ch
                )
                x1, x1v = tmp3("x1")
                nc.vector.tensor_max(x1v, M13[:, :, 0, :], M13[:, :, 1, :])
                return x1v

            if first:
                # ramp tiles: run everything U-independent (max pool +
                # vertical sum) before the conv ops so the DVE is not
                # gated on ACT's U copies while ACT is still catching up
                x1v = max_pool()
                SA = sum_part()
                zv, s4v = conv_finish(SA)
            else:
                SA = sum_part()
                zv, s4v = conv_finish(SA)
                x1v = max_pool()

            # gating: out = s4 + z*(x1 - s4)
            d, dv = tmp3("d")
            nc.vector.tensor_tensor(dv, x1v, s4v, alu.subtract)
            g, gv = tmp3("g")
            nc.vector.tensor_mul(gv, zv, dv)
            o, ov = tmp3("o", pool_out)
            nc.vector.tensor_add(ov, s4v, gv)

            # stores ride the ACT HWDGE ring: they never queue behind the
            # (much larger) input loads on the sync ring.
            dst = out_ap[b, :, q, :].rearrange("p (w c) -> p w c", c=ch)
            nc.scalar.dma_start(
                dst[:, w_lo : w_lo + nw, :],
                o[:].rearrange("p (w c) -> p w c", c=ch),
            )

        wo_q = wq // 2  # output w-pairs per quarter (32)
        n_tiles = bpc * nq
        tiles = []
        for b in range(bpc):
            for q in range(nq):
                idx = b * nq + q
                if idx == 0 and wo_q >= 8:
                    # graduated first tiles: cut the startup stall
                    tiles.append((b, q, 0, wo_q // 4))
                    tiles.append((b, q, wo_q // 4, wo_q))
                elif idx == n_tiles - 1 and wo_q >= 8:
                    # split the last tile so the final store is small
                    tiles.append((b, q, 0, 3 * wo_q // 4))
                    tiles.append((b, q, 3 * wo_q // 4, wo_q))
                else:
                    tiles.append((b, q, 0, wo_q))
        emit_warmup()
        pending = emit_load(*tiles[0])
        for i in range(len(tiles)):
            nxt = emit_load(*tiles[i + 1]) if i + 1 < len(tiles) else None
            emit_compute(pending, first=(i <= 3))
            pending = nxt

    nc.compile()
    return nc


def _get_program(bpc, ho, nq, wq, ch):
    key = (bpc, ho, nq, wq, ch)
    if key not in _PROGRAM_CACHE:
        _PROGRAM_CACHE[key] = _build_program(bpc, ho, nq, wq, ch)
    return _PROGRAM_CACHE[key]


def _mask_scalars(mask):
    """Per-partition scalar tensor [128, 8] for the conv path.

    xs = f * (u0*Ee + u1*Eo + u2*Oe + u3*Oo) with u_k = m_k / f and
    f = the mask entry of largest magnitude (signed), so |u_k| <= 1.
    f == 0 implies all-zero mask -> z = sigmoid(0) = 0.5 everywhere.
    """
    m = np.asarray(mask, np.float64).reshape(-1)  # m00, m01, m10, m11
    f = m[int(np.argmax(np.abs(m)))]
    u = m / f if f != 0.0 else np.zeros(4)
    scal = np.zeros((128, 8), np.float32)
    scal[:, 0:4] = u.astype(np.float32)
    scal[:, 4] = f
    return scal


def kernel(x, mask):
    import os

    global LAST_EXEC_NS, LAST_RESULTS

    x = np.asarray(x)
    mask = np.asarray(mask)
    assert x.shape == (B, H, W, C), x.shape
    in_dtype = x.dtype

    scal = _mask_scalars(mask)
    nc = _get_program(BPC, HO, NQ, WQ, C)

    xv = x.astype(np.float16).reshape(B, HO, 2, NQ, WQ * C)

    in_maps = [
        {"x": xv[i * BPC : (i + 1) * BPC], "scal": scal} for i in range(N_CORES)
    ]

    trace = os.environ.get("KERNEL_TRACE", "0") == "1"
    res = run_bass_kernel_spmd(
        nc, in_maps, core_ids=list(range(N_CORES)), trace=trace
    )
    LAST_EXEC_NS = res.exec_time_ns
    LAST_RESULTS = res

    parts = [
        r["out"].reshape(BPC, HO, NQ, WQ // 2, C).reshape(BPC, HO, W // 2, C)
        for r in res.results
    ]
    full = np.concatenate(parts, axis=0)
    return full.astype(np.float32, copy=False).astype(in_dtype, copy=False)


def _numpy_reference(x, mask):
    xr = x.reshape(x.shape[0], x.shape[1] // 2, 2, x.shape[2] // 2, 2, x.shape[3])
    x1 = xr.max(axis=(2, 4))
    x2 = xr.mean(axis=(2, 4))
    xs = np.einsum("bhiwjc,ij->bhwc", xr, mask)
    z = 1.0 / (1.0 + np.exp(-xs))
    return z * x1 + (1.0 - z) * x2


if __name__ == "__main__":
    # Small-scale CoreSim self-test (no hardware needed).
    from concourse.bass_interp import CoreSim

    rng = np.random.default_rng(0)
    for bpc_s, nq_s, wq_s in [(1, 1, 8), (1, 2, 32)]:
        h_s, w_s = 256, nq_s * wq_s
        xs_np = rng.standard_normal((bpc_s, h_s, w_s, C)).astype(np.float32)
        mask_np = (rng.standard_normal((2, 2)) * 0.5).astype(np.float32)

        scal_s = _mask_scalars(mask_np)
        nc = _build_program(bpc_s, 128, nq_s, wq_s, C)
        sim = CoreSim(nc, trace=False)
        xv_s = xs_np.astype(np.float16).reshape(bpc_s, 128, 2, nq_s, wq_s * C)
        sim.tensor("x")[:] = xv_s
        sim.tensor("scal")[:] = scal_s
        sim.simulate()
        got = (
            sim.tensor("out")
            .astype(np.float32)
            .reshape(bpc_s, 128, nq_s, wq_s // 2, C)
            .reshape(bpc_s, 128, w_s // 2, C)
        )
        want = _numpy_reference(
            xs_np.astype(np.float64), mask_np.astype(np.float64)
        )
        err = np.abs(got - want)
        rel = err.max() / np.abs(want).max()
        print(f"CoreSim selftest ({nq_s=} {wq_s=}): abs {err.max():.2e} rel {rel:.2e}")
        assert rel < 5e-3, rel
    print("PASS")
